# revision 14
# baseline (speedup 1.0000x reference)
"""ADGCN (3-layer GCN) distributed Bass kernel for 8 TRN2 NeuronCores.

Self-contained: hardcodes the problem shapes (N=100000, E=1600000,
512->256(relu)->64(softmax)->64) and the sharding strategy:
 - 1D node partitioning (core c owns nodes [c*12500, (c+1)*12500))
 - per layer: local dense projection, dinv-scaled feature tables
   all-gathered (two node-half collectives), per-edge dma_gather of
   message rows, segment-sum via one-hot (Sel) matmuls accumulating in
   PSUM, fused epilogues.
 - GCN self-loops are extra edges in the streams; bias enters as a
   rank-1 matmul (outer(sqrt(deg), b)) so dinv*(psum) lands exactly on
   dinv*(A^T g + g) + b.
"""

import sys

for _p in ("/opt/trn_rl_repo",):
    if _p not in sys.path:
        sys.path.insert(0, _p)

import numpy as np
from ml_dtypes import bfloat16

import concourse.bass as bass
import concourse.bacc as bacc
import concourse.mybir as mybir
import concourse.tile as tile

dt = mybir.dt
Alu = mybir.AluOpType
Act = mybir.ActivationFunctionType


# ---------------------------------------------------------------------------
# Configuration
# ---------------------------------------------------------------------------

def make_cfg(
    N=100_000,
    E=1_600_000,
    F_IN=512,
    HID=256,
    C=64,
    CORES=8,
    WINDOW=32768,
    CHUNK_TILES_L1=24,
    CHUNK_TILES_L23=24,
):
    NS = N // CORES            # nodes per core
    HALF = NS // 2             # nodes per AG half
    NCT = -(-NS // 128)        # col tiles per core
    NT = NCT * 128             # padded nodes per core
    TBL_ROWS = HALF * CORES    # rows per table (A or B)
    NWIN = -(-TBL_ROWS // WINDOW)
    NSTREAM = 2 * NWIN         # streams: (table, window)
    assert N % CORES == 0 and NS % 2 == 0
    return dict(
        N=N, E=E, F_IN=F_IN, HID=HID, C=C, CORES=CORES, WINDOW=WINDOW,
        NS=NS, HALF=HALF, NCT=NCT, NT=NT, TBL_ROWS=TBL_ROWS, NWIN=NWIN,
        NSTREAM=NSTREAM,
        CHUNK_TILES=(CHUNK_TILES_L1, CHUNK_TILES_L23, CHUNK_TILES_L23),
        TBL_F=(HID, 128, 128),   # bf16 table row widths (bytes % 256 == 0)
        AGG_F=(HID, C, C),       # real aggregated feature widths
    )


# ---------------------------------------------------------------------------
# Host preprocessing: edge plan + per-core inputs
# ---------------------------------------------------------------------------

def build_plan(cfg, edge_index):
    """Common (SPMD) static plan + per-core idx/colrel tables."""
    CORES, NS, HALF, NCT = cfg["CORES"], cfg["NS"], cfg["HALF"], cfg["NCT"]
    WINDOW, NWIN, NSTREAM = cfg["WINDOW"], cfg["NWIN"], cfg["NSTREAM"]

    row = edge_index[0].astype(np.int64)
    col = edge_index[1].astype(np.int64)
    core_of = col // NS

    def table_coords(r):
        rc = r // NS
        u = r % NS
        tbl = (u >= HALF).astype(np.int64)
        trow = HALF * rc + (u - tbl * HALF)
        return tbl * NWIN + trow // WINDOW, trow % WINDOW

    seg_counts = np.zeros((CORES, NSTREAM, NCT), dtype=np.int64)
    per_core = []
    for c in range(CORES):
        m = core_of == c
        er = np.concatenate([row[m], np.arange(NS, dtype=np.int64) + c * NS])
        ec = np.concatenate([col[m] - c * NS, np.arange(NS, dtype=np.int64)])
        s, rel = table_coords(er)
        ct, cr = ec // 128, ec % 128
        key = s * NCT + ct
        order = np.argsort(key, kind="stable")
        s, rel, ct, cr, key = s[order], rel[order], ct[order], cr[order], key[order]
        np.add.at(seg_counts[c], (s, ct), 1)
        per_core.append((s, rel, ct, cr, key))

    seg_tiles = -(-seg_counts.max(axis=0) // 128)       # [NSTREAM, NCT]
    stream_tiles = seg_tiles.sum(axis=1)
    seg_tile_start = np.cumsum(seg_tiles, axis=1) - seg_tiles
    tot_tiles = int(stream_tiles.sum())
    stream_base = np.concatenate([[0], np.cumsum(stream_tiles)])

    idx_all = np.zeros((CORES, tot_tiles * 128), dtype=np.int16)
    colrel_all = np.full((CORES, tot_tiles * 128), -1.0, dtype=np.float32)
    for c in range(CORES):
        s, rel, ct, cr, key = per_core[c]
        uniq, first_idx, counts = np.unique(
            key, return_index=True, return_counts=True)
        pos = np.arange(len(s), dtype=np.int64) - np.repeat(first_idx, counts)
        slot = (stream_base[s] + seg_tile_start[s, ct]) * 128 + pos
        idx_all[c, slot] = rel.astype(np.int16)
        colrel_all[c, slot] = cr.astype(np.float32)

    return dict(
        seg_tiles=seg_tiles, seg_tile_start=seg_tile_start,
        stream_tiles=stream_tiles, stream_base=stream_base,
        tot_tiles=tot_tiles, idx_all=idx_all,
        colrel_all=colrel_all.astype(bfloat16),
    )


def wrap_idx(idx_flat):
    """[T*128] int16 -> [128, T*8]: idx i at (i%16, i//16), replicated x8."""
    n = idx_flat.shape[0]
    w = idx_flat.reshape(n // 16, 16).T
    return np.tile(w, (8, 1)).copy()


def colrel_tiles(colrel_flat):
    """[T*128] -> [128, T]: partition = edge-in-tile, col = tile index."""
    t = colrel_flat.shape[0] // 128
    return colrel_flat.reshape(t, 128).T.copy()


def host_inputs(cfg, plan, x, edge_index, W1, b1, W2, b2, W3, b3):
    N, CORES, NS, NT, NCT = cfg["N"], cfg["CORES"], cfg["NS"], cfg["NT"], cfg["NCT"]
    F_IN, HID, C = cfg["F_IN"], cfg["HID"], cfg["C"]

    col = edge_index[1].astype(np.int64)
    deg = np.bincount(col, minlength=N).astype(np.float64) + 1.0
    dinv = (1.0 / np.sqrt(deg)).astype(np.float32)
    sqd = np.sqrt(deg).astype(np.float32)

    ident = np.eye(128, dtype=np.float32).astype(bfloat16)
    iota = np.tile(np.arange(128, dtype=np.float32), (128, 1)).astype(bfloat16)

    def ktile_pack(w):
        K, M = w.shape
        assert K % 128 == 0
        return np.concatenate(
            [w[j * 128:(j + 1) * 128] for j in range(K // 128)], axis=1)

    W1b = ktile_pack(W1.astype(np.float32)).astype(bfloat16)
    W2b = ktile_pack(W2.astype(np.float32)).astype(bfloat16)
    W3b = W3.astype(np.float32).astype(bfloat16)

    in_maps = []
    for c in range(CORES):
        xs = np.zeros((NT, F_IN), dtype=np.float32)
        xs[:NS] = x[c * NS:(c + 1) * NS]
        xTp = ktile_pack(xs.T.copy().astype(np.float32)).astype(bfloat16)

        dv = np.zeros(NT, dtype=np.float32)
        dv[:NS] = dinv[c * NS:(c + 1) * NS]
        sq = np.zeros(NT, dtype=np.float32)
        sq[:NS] = sqd[c * NS:(c + 1) * NS]

        in_maps.append({
            "xT": xTp,
            "W1": W1b, "W2": W2b, "W3": W3b,
            "b1r": b1.reshape(1, HID).astype(np.float32).astype(bfloat16),
            "b2r": b2.reshape(1, C).astype(np.float32).astype(bfloat16),
            "b3r": b3.reshape(1, C).astype(np.float32).astype(bfloat16),
            "dinv": dv.reshape(NCT, 128).T.copy(),
            "sqd": sq.reshape(1, NT).astype(bfloat16),
            "ident": ident,
            "iota": iota,
            "idx": wrap_idx(plan["idx_all"][c]),
            "colrel": colrel_tiles(plan["colrel_all"][c]),
        })
    return in_maps


# ---------------------------------------------------------------------------
# Kernel builder
# ---------------------------------------------------------------------------

def build_kernel(cfg, plan):
    NS, HALF, NCT, NT = cfg["NS"], cfg["HALF"], cfg["NCT"], cfg["NT"]
    F_IN, HID, C, CORES = cfg["F_IN"], cfg["HID"], cfg["C"], cfg["CORES"]
    TBL_ROWS, WINDOW, NWIN, NSTREAM = (
        cfg["TBL_ROWS"], cfg["WINDOW"], cfg["NWIN"], cfg["NSTREAM"])
    TBL_F, AGG_F, CHUNKS = cfg["TBL_F"], cfg["AGG_F"], cfg["CHUNK_TILES"]
    KI, KH = F_IN // 128, HID // 128

    seg_tiles = plan["seg_tiles"]
    seg_tile_start = plan["seg_tile_start"]
    stream_tiles = plan["stream_tiles"]
    stream_base = plan["stream_base"]
    tot_tiles = plan["tot_tiles"]

    nc = bacc.Bacc(None, target_bir_lowering=False,
                   num_swdge_queues=4,
                   dynamic_dma_scratch_size=cfg.get("DMA_SCRATCH", 16384))

    xT_ext = nc.declare_dram_parameter("xT", [128, KI * NT], dt.bfloat16, isOutput=False)
    W1_ext = nc.declare_dram_parameter("W1", [128, KI * HID], dt.bfloat16, isOutput=False)
    W2_ext = nc.declare_dram_parameter("W2", [128, KH * C], dt.bfloat16, isOutput=False)
    W3_ext = nc.declare_dram_parameter("W3", [C, C], dt.bfloat16, isOutput=False)
    b_ext = [
        nc.declare_dram_parameter("b1r", [1, HID], dt.bfloat16, isOutput=False),
        nc.declare_dram_parameter("b2r", [1, C], dt.bfloat16, isOutput=False),
        nc.declare_dram_parameter("b3r", [1, C], dt.bfloat16, isOutput=False),
    ]
    dinv_ext = nc.declare_dram_parameter("dinv", [128, NCT], dt.float32, isOutput=False)
    sqd_ext = nc.declare_dram_parameter("sqd", [1, NT], dt.bfloat16, isOutput=False)
    ident_ext = nc.declare_dram_parameter("ident", [128, 128], dt.bfloat16, isOutput=False)
    iota_ext = nc.declare_dram_parameter("iota", [128, 128], dt.bfloat16, isOutput=False)
    idx_ext = nc.declare_dram_parameter("idx", [128, tot_tiles * 8], dt.int16, isOutput=False)
    colrel_ext = nc.declare_dram_parameter("colrel", [128, tot_tiles], dt.bfloat16, isOutput=False)
    out_ext = nc.declare_dram_parameter("out", [NS, C], dt.float32, isOutput=True)

    bounce, tables = [], []
    for li, tf in enumerate(TBL_F):
        ba = nc.dram_tensor(f"g{li}_a", [HALF, tf], dt.bfloat16)
        bb = nc.dram_tensor(f"g{li}_b", [HALF, tf], dt.bfloat16)
        ta = nc.dram_tensor(f"tbl{li}_a", [TBL_ROWS, tf], dt.bfloat16, addr_space="Shared")
        tb = nc.dram_tensor(f"tbl{li}_b", [TBL_ROWS, tf], dt.bfloat16, addr_space="Shared")
        bounce.append((ba, bb))
        tables.append((ta, tb))

    replica_groups = [list(range(CORES))]

    with tile.TileContext(nc) as tc:
        with (
            tc.tile_pool(name="const", bufs=1) as cpool,
            tc.tile_pool(name="work", bufs=3) as wpool,
            tc.tile_pool(name="small", bufs=4) as spool,
            tc.tile_pool(name="idxp", bufs=3) as idxpool,
        ):
            from concourse import library_config
            nc.gpsimd.load_library(library_config.attnmlp)

            # ---- constants -------------------------------------------
            W1_sb = cpool.tile([128, KI * HID], dt.bfloat16, tag="W1")
            W2_sb = cpool.tile([128, KH * C], dt.bfloat16, tag="W2")
            W3_sb = cpool.tile([C, C], dt.bfloat16, tag="W3")
            b_sb = [
                cpool.tile([1, HID], dt.bfloat16, tag="b1", name="b1_sb"),
                cpool.tile([1, C], dt.bfloat16, tag="b2", name="b2_sb"),
                cpool.tile([1, C], dt.bfloat16, tag="b3", name="b3_sb"),
            ]
            dinv_sb = cpool.tile([128, NCT], dt.float32, tag="dinv")
            sqd_sb = cpool.tile([1, NT], dt.bfloat16, tag="sqd")
            ident_sb = cpool.tile([128, 128], dt.bfloat16, tag="ident")
            iota_sb = cpool.tile([128, 128], dt.bfloat16, tag="iota")
            colrel_sb = cpool.tile([128, tot_tiles], dt.bfloat16, tag="colrel")

            nc.sync.dma_start(W1_sb[:, :], W1_ext[:, :])
            nc.sync.dma_start(W2_sb[:, :], W2_ext[:, :])
            nc.sync.dma_start(W3_sb[:, :], W3_ext[:, :])
            for bs, be in zip(b_sb, b_ext):
                nc.sync.dma_start(bs[:, :], be[:, :])
            nc.sync.dma_start(dinv_sb[:, :], dinv_ext[:, :])
            nc.sync.dma_start(sqd_sb[:, :], sqd_ext[:, :])
            nc.sync.dma_start(ident_sb[:, :], ident_ext[:, :])
            nc.sync.dma_start(iota_sb[:, :], iota_ext[:, :])
            nc.sync.dma_start(colrel_sb[:, :], colrel_ext[:, :])

            def dma_rows_to_halves(li, src_tile, ct, fwidth):
                ba, bb = bounce[li]
                r0, r1 = ct * 128, min(ct * 128 + 128, NS)
                if r1 <= r0:
                    return
                if r0 < HALF:
                    e = min(r1, HALF)
                    nc.sync.dma_start(ba[r0:e, :fwidth], src_tile[0:e - r0, :fwidth])
                if r1 > HALF:
                    s0 = max(r0, HALF)
                    nc.sync.dma_start(
                        bb[s0 - HALF:r1 - HALF, :fwidth],
                        src_tile[s0 - r0:r1 - r0, :fwidth])

            def all_gathers(li):
                ba, bb = bounce[li]
                ta, tb = tables[li]
                for src, dst in ((ba, ta), (bb, tb)):
                    nc.gpsimd.collective_compute(
                        "AllGather", Alu.bypass,
                        replica_groups=replica_groups,
                        ins=[src[:, :]], outs=[dst[:, :]],
                    )

            # ---- Phase 1: h1 = xT.T @ W1 tiles; g1 = dinv * h1 -------
            with (
                tc.tile_pool(name="xt", bufs=1) as xtp,
                tc.tile_pool(name="psm1", bufs=2, space="PSUM") as pspool,
            ):
                xT_sb = xtp.tile([128, KI * NT], dt.bfloat16, tag="xT")
                nc.sync.dma_start(xT_sb[:, :], xT_ext[:, :])
                for ct in range(NCT):
                    ps = pspool.tile([128, HID], dt.float32, tag="mm1")
                    for k in range(KI):
                        nc.tensor.matmul(
                            ps[:, :],
                            xT_sb[:, k * NT + ct * 128:k * NT + (ct + 1) * 128],
                            W1_sb[:, k * HID:(k + 1) * HID],
                            start=(k == 0), stop=(k == KI - 1))
                    g1 = wpool.tile([128, HID], dt.bfloat16, tag="g1")
                    nc.vector.tensor_scalar(
                        g1[:, :], ps[:, :], dinv_sb[:, ct:ct + 1], None, Alu.mult)
                    dma_rows_to_halves(0, g1, ct, HID)
            all_gathers(0)

            # ---- SpMM machinery --------------------------------------
            def spmm_layer(li, consume):
                tf, af, ctiles = TBL_F[li], AGG_F[li], CHUNKS[li]
                ta, tb = tables[li]
                msg_tiles, sel_tiles = {}, {}
                with (
                    tc.tile_pool(name=f"msg{li}", bufs=2) as mpool,
                    tc.tile_pool(name=f"sel{li}", bufs=2) as selpool,
                    tc.tile_pool(name=f"psA{li}", bufs=2, space="PSUM") as pspool,
                    tc.tile_pool(name=f"psB{li}", bufs=2, space="PSUM") as ps2pool,
                ):
                    for s in range(NSTREAM):
                        tbl = ta if s < NWIN else tb
                        win = s % NWIN
                        rows0 = win * WINDOW
                        rows1 = min(rows0 + WINDOW, TBL_ROWS)
                        st = int(stream_tiles[s])
                        gb = int(stream_base[s])
                        for kch in range(-(-st // ctiles)):
                            t0, t1 = kch * ctiles, min((kch + 1) * ctiles, st)
                            ntile = t1 - t0
                            nidx = ntile * 128
                            it = idxpool.tile(
                                [128, ntile * 8], dt.int16, tag=f"idx{s}")
                            nc.sync.dma_start(
                                it[:, :],
                                idx_ext[:, (gb + t0) * 8:(gb + t1) * 8])
                            mt = mpool.tile(
                                [128, ntile * tf], dt.bfloat16, tag=f"m{s}")
                            nc.gpsimd.dma_gather(
                                out_ap=mt[:, :].rearrange(
                                    "p (t e) -> p t e", e=tf),
                                in_ap=tbl[rows0:rows1, :],
                                idxs_ap=it[:, :],
                                num_idxs=nidx,
                                num_idxs_reg=nidx,
                                elem_size=tf,
                                queue_num=s % 4,
                                single_packet=False,
                            )
                            se = selpool.tile(
                                [128, ntile * 128], dt.bfloat16, tag=f"s{s}")
                            i3 = iota_sb[:, :].rearrange("p (o j) -> p o j", o=1)
                            c3 = colrel_sb[:, gb + t0:gb + t1].rearrange(
                                "p (t o) -> p t o", o=1)
                            i3b, c3b = bass.broadcast_tensor_aps(i3, c3)
                            nc.vector.tensor_tensor(
                                se[:, :].rearrange("p (t j) -> p t j", j=128),
                                i3b, c3b, Alu.is_equal)
                            msg_tiles[(s, kch)] = mt
                            sel_tiles[(s, kch)] = se

                    for ct in range(NCT):
                        ps = pspool.tile([128, af], dt.float32, tag=f"agg{li}")
                        nc.tensor.matmul(
                            ps[:, :],
                            sqd_sb[0:1, ct * 128:(ct + 1) * 128],
                            b_sb[li][0:1, :af],
                            start=True, stop=False)
                        tot_ct = int(seg_tiles[:, ct].sum())
                        ndone = 0
                        for s in range(NSTREAM):
                            t0 = int(seg_tile_start[s, ct])
                            for j in range(int(seg_tiles[s, ct])):
                                gt = t0 + j
                                kch, slot = gt // ctiles, gt % ctiles
                                mt = msg_tiles[(s, kch)]
                                se = sel_tiles[(s, kch)]
                                ndone += 1
                                nc.tensor.matmul(
                                    ps[:, :],
                                    se[:, slot * 128:(slot + 1) * 128],
                                    mt[:, slot * tf:slot * tf + af],
                                    start=False, stop=(ndone == tot_ct))
                        consume(ct, ps, ps2pool)

            # ---- Layer 1 consume: relu, transpose, M2, g2 ------------
            def consume_l1(ct, ps, ps2pool):
                h1 = wpool.tile([128, HID], dt.bfloat16, tag="h1")
                nc.vector.tensor_scalar(
                    h1[:, :], ps[:, :], dinv_sb[:, ct:ct + 1], 0.0,
                    Alu.mult, Alu.max)
                pt = ps2pool.tile([128, HID], dt.bfloat16, tag="tr1")
                for k in range(KH):
                    nc.tensor.transpose(
                        pt[:, k * 128:(k + 1) * 128],
                        h1[:, k * 128:(k + 1) * 128], ident_sb[:, :])
                h1T = wpool.tile([128, HID], dt.bfloat16, tag="h1T")
                nc.vector.tensor_copy(h1T[:, :], pt[:, :])
                ps2 = ps2pool.tile([128, C], dt.float32, tag="mm2")
                for k in range(KH):
                    nc.tensor.matmul(
                        ps2[:, :], h1T[:, k * 128:(k + 1) * 128],
                        W2_sb[:, k * C:(k + 1) * C],
                        start=(k == 0), stop=(k == KH - 1))
                g2 = wpool.tile([128, 128], dt.bfloat16, tag="g2")
                nc.vector.memset(g2[:, C:128], 0.0)
                nc.vector.tensor_scalar(
                    g2[:, 0:C], ps2[:, :], dinv_sb[:, ct:ct + 1], None, Alu.mult)
                dma_rows_to_halves(1, g2, ct, 128)

            spmm_layer(0, consume_l1)
            all_gathers(1)

            # ---- Layer 2 consume: softmax, transpose, M3, g3 ---------
            def consume_l2(ct, ps, ps2pool):
                h2 = spool.tile([128, C], dt.float32, tag="h2")
                nc.vector.tensor_scalar(
                    h2[:, :], ps[:, :], dinv_sb[:, ct:ct + 1], None, Alu.mult)
                nmax = spool.tile([128, 1], dt.float32, tag="nmax")
                nc.vector.tensor_reduce(
                    nmax[:, :], h2[:, :], mybir.AxisListType.X, Alu.max,
                    negate=True)
                ex = spool.tile([128, C], dt.float32, tag="ex")
                sm = spool.tile([128, 1], dt.float32, tag="sm")
                nc.scalar.activation(
                    ex[:, :], h2[:, :], Act.Exp, bias=nmax[:, :],
                    accum_out=sm[:, :])
                rec = spool.tile([128, 1], dt.float32, tag="rec")
                nc.vector.reciprocal(rec[:, :], sm[:, :])
                h2s = spool.tile([128, C], dt.bfloat16, tag="h2s")
                nc.vector.tensor_scalar(
                    h2s[:, :], ex[:, :], rec[:, :], None, Alu.mult)
                pt = ps2pool.tile([C, 128], dt.bfloat16, tag="tr2")
                nc.tensor.transpose(pt[:, :], h2s[:, :], ident_sb[:, :])
                h2sT = spool.tile([C, 128], dt.bfloat16, tag="h2sT")
                nc.vector.tensor_copy(h2sT[:, :], pt[:, :])
                ps3 = ps2pool.tile([128, C], dt.float32, tag="mm3")
                nc.tensor.matmul(
                    ps3[:, :], h2sT[:, :], W3_sb[:, :], start=True, stop=True)
                g3 = wpool.tile([128, 128], dt.bfloat16, tag="g3")
                nc.vector.memset(g3[:, C:128], 0.0)
                nc.vector.tensor_scalar(
                    g3[:, 0:C], ps3[:, :], dinv_sb[:, ct:ct + 1], None, Alu.mult)
                dma_rows_to_halves(2, g3, ct, 128)

            spmm_layer(1, consume_l2)
            all_gathers(2)

            # ---- Layer 3 consume: final epilogue + output ------------
            def consume_l3(ct, ps, ps2pool):
                ot = spool.tile([128, C], dt.float32, tag="ot")
                nc.vector.tensor_scalar(
                    ot[:, :], ps[:, :], dinv_sb[:, ct:ct + 1], None, Alu.mult)
                r0, r1 = ct * 128, min(ct * 128 + 128, NS)
                if r1 > r0:
                    nc.sync.dma_start(out_ext[r0:r1, :], ot[0:r1 - r0, :])

            spmm_layer(2, consume_l3)

    nc.finalize()
    return nc


# ---------------------------------------------------------------------------
# Public entry point
# ---------------------------------------------------------------------------

_CACHE = {}


def _get_built(cfg, edge_index):
    key = ("k", edge_index.tobytes()[:64], edge_index.shape)
    if key not in _CACHE:
        plan = build_plan(cfg, edge_index)
        nc = build_kernel(cfg, plan)
        _CACHE[key] = (plan, nc)
    return _CACHE[key]


def kernel(x, edge_index, W1, b1, W2, b2, W3, b3):
    from concourse.bass_utils import run_bass_kernel_spmd

    cfg = make_cfg()
    x = np.asarray(x, dtype=np.float32)
    edge_index = np.asarray(edge_index)
    plan, nc = _get_built(cfg, edge_index)
    in_maps = host_inputs(cfg, plan, x, edge_index,
                          np.asarray(W1), np.asarray(b1), np.asarray(W2),
                          np.asarray(b2), np.asarray(W3), np.asarray(b3))
    res = run_bass_kernel_spmd(nc, in_maps, core_ids=list(range(cfg["CORES"])))
    out = np.concatenate(
        [res.results[i]["out"] for i in range(cfg["CORES"])], axis=0)
    return out[:cfg["N"]].astype(np.float32)
